# revision 15
# baseline (speedup 1.0000x reference)
"""Graph-LSTM encoder kernel for 8x Trainium2 NeuronCores.

Problem: B,T,N,F,H = 64,50,24,256,256
    h = graph_linear(G, x0, W_h1, b_h1); c = graph_linear(G, x0, W_h2, b_h2)
    per t: gates = GL(G, x_t, W_ih, b_ih) + GL(G, h, W_hh, b_hh)  (LSTM cell)
    out = tanh(GL(G, h_T, W_fc, b_fc))
where GL(G, x, W, b) = einsum('nm,bmf->bnf', G, x @ W.T) + b
                     = (G . x) @ W.T + b      (mix commutes with projection)

Sharding: data-parallel over batch, 8 batches/core. Per core, batches are
split into 2 groups of 4 (96 rows of 24 nodes each) which pipeline against
each other (PE on one group while ACT/DVE handle the other).

Layouts per group (rows = (batch-in-group, node) on partitions):
  state   h [96,256] bf16, c [96,256] f32 (persistent SBUF)
  mix     mm(lhsT=z[96, fc*128:+128], rhs=BD[96,96]) -> psum[128, fc*128:+96]
          where BD = kron(I4, G^T): block-diag node mix, contracted over rows.
  gates   [128,1024] psum (rows 96+ garbage), accumulated as
            ones[1,128]^T @ bias[1,512]        (bias row, start=True)
          + GzT blocks ^T @ W blocks           (bf16, M=128 for fast LDW)
  cell    gates pre-permuted to [g,i | f,o]: one tanh(g) (can start when
          psum bank0 closes), ONE merged sigmoid over [i,f,o] (768 cols),
          then bf16 DVE cell math. All mix-psum casts run on the idle
          GPSIMD engine so the DVE only does cell arithmetic.

Software pipeline (per loop iteration s, group g = s%2):
  PE:     h-mix(s) | bias+x-gates(s+2) | x-mix(s+4) | h-gates(s)
  GPSIMD: h-mix cast(s), x-mix cast(s+4)
  Sync:   x DMA(s+6)
  ACT/DVE: cell(s)
Each stage consumes data produced >= 2 iterations earlier, so no
intra-iteration cross-engine stalls on the PE queue.

Weights arrive in 2 packed DMAs (init-critical small blob first), not 15.
"""

import sys

sys.path.insert(0, "/opt/trn_rl_repo")

import numpy as np
import ml_dtypes

import concourse.bacc as bacc
import concourse.mybir as mybir
import concourse.tile as tile
from concourse.bass_utils import run_bass_kernel_spmd

B, T, N, F, H = 64, 50, 24, 256, 256
NCORES = 8
B_LOC = B // NCORES      # 8 batches per core
NG = 2                   # pipeline groups per core
BG = B_LOC // NG         # 4 batches per group
R = BG * N               # 96 rows per group
G4 = 4 * H               # 1024 gate width
NSTEP = T * NG

F32 = mybir.dt.float32
BF16 = mybir.dt.bfloat16

LAST_EXEC_NS = None
RUN_KWARGS = {}

# --- packed weight blob column offsets (bf16, [128, *]) ---
# small (init-critical): bd | w1 | w2 | ones | b1 | b2
C_BD, C_W1, C_W2, C_ONES, C_B1, C_B2, C_SMALL = 0, 96, 608, 1120, 1248, 1504, 1760
# big: wih | whh | wfc | biasg | bfc
C_WIH, C_WHH, C_WFC, C_BG, C_BFC, C_BIG = 0, 2048, 4096, 4608, 5632, 5888


def _perm_ifog(a, axis=0):
    """[i,f,g,o] -> [i,g,f,o]: bank0 = (i,g) closes first and feeds the
    cell (sig_i, tanh_g, m1) while the PE still fills bank1 = (f,o)."""
    idx = np.concatenate([
        np.arange(0, H),          # i
        np.arange(2 * H, 3 * H),  # g
        np.arange(H, 2 * H),      # f
        np.arange(3 * H, 4 * H),  # o
    ])
    return np.take(a, idx, axis=axis)


def _build_bass():
    nc = bacc.Bacc("TRN2", target_bir_lowering=False, debug=False)

    x_ext = nc.declare_dram_parameter("x", [T, NG, R, F], BF16, isOutput=False)
    ws_ext = nc.declare_dram_parameter("ws", [128, C_SMALL], BF16, isOutput=False)
    wb_ext = nc.declare_dram_parameter("wb", [128, C_BIG], BF16, isOutput=False)
    out_ext = nc.declare_dram_parameter("out", [NG, R, H], F32, isOutput=True)

    with tile.TileContext(nc) as tc:
        with (
            tc.tile_pool(name="wpool", bufs=1) as wpool,
            tc.tile_pool(name="state", bufs=1) as state,
            tc.tile_pool(name="xpool", bufs=4) as xpool,
            tc.tile_pool(name="mixps", bufs=1, space="PSUM") as mixps,
            tc.tile_pool(name="sbx", bufs=4) as sbx,
            tc.tile_pool(name="sbh", bufs=2) as sbh,
            tc.tile_pool(name="gps", bufs=3, space="PSUM") as gps,
            tc.tile_pool(name="ew", bufs=2) as ew,
        ):
            # ---- weights: two packed DMAs (init-critical blob first) ----
            ws = wpool.tile([128, C_SMALL], BF16)
            nc.sync.dma_start(ws[:], ws_ext[:])
            wb = wpool.tile([128, C_BIG], BF16)
            nc.sync.dma_start(wb[:], wb_ext[:])

            bd = ws[0:96, C_BD:C_BD + 96]
            w1 = ws[:, C_W1:C_W2]
            w2 = ws[:, C_W2:C_ONES]
            ones = ws[0:1, C_ONES:C_B1]
            b1 = ws[0:1, C_B1:C_B2]
            b2 = ws[0:1, C_B2:C_SMALL]
            wih = wb[:, C_WIH:C_WHH]
            whh = wb[:, C_WHH:C_WFC]
            wfc = wb[:, C_WFC:C_BG]
            biasg = wb[0:1, C_BG:C_BFC]
            bfc = wb[0:1, C_BFC:C_BIG]

            # ---- x tiles: DMA the first 4 steps up-front ----
            xts = {}

            def stage_dma(s):
                xt = xpool.tile([R, F], BF16, tag="xt", name="xt")
                nc.sync.dma_start(xt[:], x_ext[s // NG, s % NG])
                xts[s] = xt

            for s in range(4):
                stage_dma(s)

            # PE warm-up: keep the PE busy through the weight-DMA window so
            # the clock gate opens and the pstate ramps before step 0.
            for wtag in ("mpx", "mph"):
                wu_ps = mixps.tile([128, 256], F32, tag=wtag, name=f"wu_{wtag}")
                for _ in range(30):
                    nc.tensor.matmul(wu_ps[0:R, 0:R], bd, bd,
                                     start=True, stop=True)

            # ---- persistent state ----
            hs = [state.tile([R, H], BF16, tag=f"h{g}", name=f"h{g}")
                  for g in range(NG)]
            cs = [state.tile([R, H], BF16, tag=f"c{g}", name=f"c{g}")
                  for g in range(NG)]

            msx = {}

            def stage_xmix(s):
                """x-mix for step s: PE mix matmuls + GPSIMD psum->sbuf cast."""
                xt = xts.pop(s)
                ps = mixps.tile([128, 256], F32, tag="mpx", name="mpx")
                for fc in range(2):
                    nc.tensor.matmul(
                        ps[:, fc * 128:fc * 128 + R],
                        xt[:, fc * 128:(fc + 1) * 128],
                        bd, start=True, stop=True)
                sb = sbx.tile([128, 256], BF16, tag="msx", name="msx")
                ps_v = ps[:].rearrange("p (c k) -> p c k", k=128)[:, :, 0:R]
                sb_v = sb[:].rearrange("p (c k) -> p c k", k=128)[:, :, 0:R]
                nc.vector.tensor_copy(sb_v, ps_v)
                msx[s] = sb

            def mix_h(hsb):
                """h-mix: PE matmuls + GPSIMD cast (on the recurrence path)."""
                ps = mixps.tile([128, 256], F32, tag="mph", name="mph")
                for fc in range(2):
                    nc.tensor.matmul(
                        ps[:, fc * 128:fc * 128 + R],
                        hsb[:, fc * 128:(fc + 1) * 128],
                        bd, start=True, stop=True)
                sb = sbh.tile([128, 256], BF16, tag="msh", name="msh")
                ps_v = ps[:].rearrange("p (c k) -> p c k", k=128)[:, :, 0:R]
                sb_v = sb[:].rearrange("p (c k) -> p c k", k=128)[:, :, 0:R]
                nc.vector.tensor_copy(sb_v, ps_v)
                return sb

            pending = {}

            def stage_bias(s):
                """open step s's gates psum with the bias rows."""
                ps = gps.tile([128, G4], F32, tag="gates", name="gates")
                for nch in range(2):
                    nc.tensor.matmul(ps[:, nch * 512:(nch + 1) * 512],
                                     ones, biasg[:, nch * 512:(nch + 1) * 512],
                                     start=True, stop=False)
                pending[s] = ps

            def stage_xg(s):
                """x-side matmuls into step s's gates psum."""
                gxT = msx.pop(s)
                ps = pending[s]
                for nch in range(2):
                    for fc in range(2):
                        nc.tensor.matmul(
                            ps[:, nch * 512:(nch + 1) * 512],
                            gxT[:, fc * 128:(fc + 1) * 128],
                            wih[:, fc * G4 + nch * 512:fc * G4 + (nch + 1) * 512],
                            start=False, stop=False)

            def close_gates(ps, ghT):
                """h-side matmuls; nch0 (psum bank0 = g,i) stops first."""
                for nch in range(2):
                    for fc in range(2):
                        nc.tensor.matmul(
                            ps[:, nch * 512:(nch + 1) * 512],
                            ghT[:, fc * 128:(fc + 1) * 128],
                            whh[:, fc * G4 + nch * 512:fc * G4 + (nch + 1) * 512],
                            start=False, stop=(fc == 1))

            hmul_pending = {}

            def cell(s):
                """Everything except the final h = sig_o * tanh(c), which is
                issued next iteration (flush_hmul) so the next h-mix cast is
                not stuck behind it in the DVE queue."""
                g = s % NG
                ps = pending.pop(s)
                # bank0 = (i,g): starts as soon as the first psum bank closes
                sig_i = ew.tile([R, H], BF16, tag="sigi", name="sig_i")
                nc.scalar.activation(sig_i[:], ps[0:R, 0:H],
                                     mybir.ActivationFunctionType.Sigmoid)
                tg = ew.tile([R, H], BF16, tag="tg", name="tg")
                nc.scalar.activation(tg[:], ps[0:R, H:2 * H],
                                     mybir.ActivationFunctionType.Tanh)
                m1 = ew.tile([R, H], BF16, tag="m1", name="m1")
                nc.vector.tensor_mul(m1[:], sig_i[:], tg[:])
                # bank1 = (f,o)
                sig_fo = ew.tile([R, 2 * H], BF16, tag="sigfo", name="sig_fo")
                nc.scalar.activation(sig_fo[:], ps[0:R, 2 * H:4 * H],
                                     mybir.ActivationFunctionType.Sigmoid)
                m2 = ew.tile([R, H], BF16, tag="m2", name="m2")
                nc.vector.tensor_mul(m2[:], sig_fo[:, 0:H], cs[g][:])
                nc.vector.tensor_add(cs[g][:], m1[:], m2[:])
                tc_t = ew.tile([R, H], BF16, tag="tc", name="tc")
                nc.scalar.activation(tc_t[:], cs[g][:],
                                     mybir.ActivationFunctionType.Tanh)
                nc.vector.tensor_mul(hs[g][:], sig_fo[:, H:2 * H], tc_t[:])

            def flush_hmul(s):
                pass

            def proj_init(gxT, w_t, bias_t, dst):
                """dst[96,256] = (gxT^T stacked) @ w + bias (init h0/c0)."""
                ps = gps.tile([128, G4], F32, tag="gates", name="gates")
                nc.tensor.matmul(ps[:, 0:H], ones, bias_t,
                                 start=True, stop=False)
                for fc in range(2):
                    nc.tensor.matmul(
                        ps[:, 0:H],
                        gxT[:, fc * 128:(fc + 1) * 128],
                        w_t[:, fc * H:(fc + 1) * H],
                        start=False, stop=(fc == 1))
                nc.vector.tensor_copy(dst[:], ps[0:R, 0:H])

            # ---- prologue: init h0/c0 (reuses step-0/1 x-mixes) ----
            stage_xmix(0)
            stage_xmix(1)
            for g in range(NG):
                proj_init(msx[g], w1, b1, hs[g])
                proj_init(msx[g], w2, b2, cs[g])
            stage_dma(4)
            stage_dma(5)
            stage_xmix(2)
            stage_xmix(3)
            stage_bias(0)
            stage_xg(0)

            # ---- recurrence ----
            for s in range(NSTEP):
                g = s % NG
                ghT = mix_h(hs[g])
                if s + 1 < NSTEP:
                    stage_bias(s + 1)
                if s + 4 < NSTEP:
                    stage_xmix(s + 4)
                if s + 6 < NSTEP:
                    stage_dma(s + 6)
                if s + 1 < NSTEP:
                    stage_xg(s + 1)
                close_gates(pending[s], ghT)
                cell(s)

            # ---- final projection ----
            for g in range(NG):
                ghT = mix_h(hs[g])
                ps = gps.tile([128, G4], F32, tag="gates", name="gates")
                nc.tensor.matmul(ps[:, 0:H], ones, bfc, start=True, stop=False)
                for fc in range(2):
                    nc.tensor.matmul(
                        ps[:, 0:H],
                        ghT[:, fc * 128:(fc + 1) * 128],
                        wfc[:, fc * H:(fc + 1) * H],
                        start=False, stop=(fc == 1))
                o_sb = ew.tile([R, H], F32, tag="osb", name="osb")
                nc.scalar.activation(o_sb[:], ps[0:R, 0:H],
                                     mybir.ActivationFunctionType.Tanh)
                nc.sync.dma_start(out_ext[g], o_sb[:])

    nc.compile()
    return nc


_NC_CACHE = None


def kernel(x, G, W_ih, b_ih, W_hh, b_hh, W_h1, b_h1, W_h2, b_h2, W_fc, b_fc):
    global _NC_CACHE, LAST_EXEC_NS

    x = np.asarray(x)
    G = np.asarray(G, dtype=np.float32)

    # host-side staging
    # x: [B,T,N,F] -> per-core [T, NG, R, F] with b = core*B_LOC + g*BG + bb
    xs = np.asarray(x, dtype=np.float32).reshape(NCORES, NG, BG, T, N, F)
    xs = xs.transpose(0, 3, 1, 2, 4, 5).reshape(NCORES, T, NG, R, F)
    xs = xs.astype(ml_dtypes.bfloat16)

    bd = np.kron(np.eye(BG, dtype=np.float32), G.T)

    def _wt(w):  # [out, in] -> lhs-side [128, 2*out] (feat chunks along cols)
        wt = np.ascontiguousarray(np.asarray(w, np.float32).T)  # [in, out]
        return np.concatenate([wt[0:128], wt[128:256]], axis=1)

    ws = np.zeros((128, C_SMALL), np.float32)
    ws[0:96, C_BD:C_BD + 96] = bd
    ws[:, C_W1:C_W2] = _wt(W_h1)
    ws[:, C_W2:C_ONES] = _wt(W_h2)
    ws[0, C_ONES:C_B1] = 1.0
    ws[0, C_B1:C_B2] = np.asarray(b_h1, np.float32)
    ws[0, C_B2:C_SMALL] = np.asarray(b_h2, np.float32)

    wbig = np.zeros((128, C_BIG), np.float32)
    wbig[:, C_WIH:C_WHH] = _wt(_perm_ifog(np.asarray(W_ih)))
    wbig[:, C_WHH:C_WFC] = _wt(_perm_ifog(np.asarray(W_hh)))
    wbig[:, C_WFC:C_BG] = _wt(W_fc)
    wbig[0, C_BG:C_BFC] = _perm_ifog(
        np.asarray(b_ih, np.float32) + np.asarray(b_hh, np.float32))
    wbig[0, C_BFC:C_BIG] = np.asarray(b_fc, np.float32)

    ws = ws.astype(ml_dtypes.bfloat16)
    wbig = wbig.astype(ml_dtypes.bfloat16)

    if _NC_CACHE is None:
        _NC_CACHE = _build_bass()
    nc = _NC_CACHE

    in_maps = [dict(x=xs[core], ws=ws, wb=wbig) for core in range(NCORES)]

    res = run_bass_kernel_spmd(nc, in_maps, list(range(NCORES)), **RUN_KWARGS)
    LAST_EXEC_NS = res.exec_time_ns

    out = np.empty((B, N, H), np.float32)
    for core in range(NCORES):
        o = res.results[core]["out"].reshape(NG, BG, N, H)
        for g in range(NG):
            for bb in range(BG):
                out[core * B_LOC + g * BG + bb] = o[g, bb]
    return out


if __name__ == "__main__":
    rng = np.random.default_rng(0)
    ins = {
        "x": rng.standard_normal((B, T, N, F), np.float32),
        "G": rng.standard_normal((N, N), np.float32) / np.sqrt(N),
        "W_ih": rng.standard_normal((G4, F), np.float32) * 0.05,
        "b_ih": rng.standard_normal((G4,), np.float32) * 0.05,
        "W_hh": rng.standard_normal((G4, H), np.float32) * 0.05,
        "b_hh": rng.standard_normal((G4,), np.float32) * 0.05,
        "W_h1": rng.standard_normal((H, F), np.float32) * 0.05,
        "b_h1": rng.standard_normal((H,), np.float32) * 0.05,
        "W_h2": rng.standard_normal((H, F), np.float32) * 0.05,
        "b_h2": rng.standard_normal((H,), np.float32) * 0.05,
        "W_fc": rng.standard_normal((H, H), np.float32) * 0.05,
        "b_fc": rng.standard_normal((H,), np.float32) * 0.05,
    }
    out = kernel(**ins)
    print("out", out.shape, out.dtype, float(np.abs(out).mean()))


# revision 16
# speedup vs baseline: 1.1805x; 1.1805x over previous
"""Graph-LSTM encoder kernel for 8x Trainium2 NeuronCores.

Problem: B,T,N,F,H = 64,50,24,256,256
    h = graph_linear(G, x0, W_h1, b_h1); c = graph_linear(G, x0, W_h2, b_h2)
    per t: gates = GL(G, x_t, W_ih, b_ih) + GL(G, h, W_hh, b_hh)  (LSTM cell)
    out = tanh(GL(G, h_T, W_fc, b_fc))
where GL(G, x, W, b) = einsum('nm,bmf->bnf', G, x @ W.T) + b
                     = (G . x) @ W.T + b      (mix commutes with projection)

Sharding: data-parallel over batch, 8 batches/core. Per core, batches are
split into 2 groups of 4 (96 rows of 24 nodes each) which pipeline against
each other (PE on one group while ACT/DVE handle the other).

Layouts per group (rows = (batch-in-group, node) on partitions):
  state   h [96,256] bf16, c [96,256] f32 (persistent SBUF)
  mix     mm(lhsT=z[96, fc*128:+128], rhs=BD[96,96]) -> psum[128, fc*128:+96]
          where BD = kron(I4, G^T): block-diag node mix, contracted over rows.
  gates   [128,1024] psum (rows 96+ garbage), accumulated as
            ones[1,128]^T @ bias[1,512]        (bias row, start=True)
          + GzT blocks ^T @ W blocks           (bf16, M=128 for fast LDW)
  cell    gates pre-permuted to [g,i | f,o]: one tanh(g) (can start when
          psum bank0 closes), ONE merged sigmoid over [i,f,o] (768 cols),
          then bf16 DVE cell math. All mix-psum casts run on the idle
          GPSIMD engine so the DVE only does cell arithmetic.

Software pipeline (per loop iteration s, group g = s%2):
  PE:     h-mix(s) | bias+x-gates(s+2) | x-mix(s+4) | h-gates(s)
  GPSIMD: h-mix cast(s), x-mix cast(s+4)
  Sync:   x DMA(s+6)
  ACT/DVE: cell(s)
Each stage consumes data produced >= 2 iterations earlier, so no
intra-iteration cross-engine stalls on the PE queue.

Weights arrive in 2 packed DMAs (init-critical small blob first), not 15.
"""

import sys

sys.path.insert(0, "/opt/trn_rl_repo")

import numpy as np
import ml_dtypes

import concourse.bacc as bacc
import concourse.mybir as mybir
import concourse.tile as tile
from concourse.bass_utils import run_bass_kernel_spmd

B, T, N, F, H = 64, 50, 24, 256, 256
NCORES = 8
B_LOC = B // NCORES      # 8 batches per core
NG = 2                   # pipeline groups per core
BG = B_LOC // NG         # 4 batches per group
R = BG * N               # 96 rows per group
G4 = 4 * H               # 1024 gate width
NSTEP = T * NG

F32 = mybir.dt.float32
BF16 = mybir.dt.bfloat16

LAST_EXEC_NS = None
RUN_KWARGS = {}

# --- packed weight blob column offsets (bf16, [128, *]) ---
# small (init-critical): bd | w1 | w2 | ones | b1 | b2
C_BD, C_W1, C_W2, C_ONES, C_B1, C_B2, C_SMALL = 0, 96, 608, 1120, 1248, 1504, 1760
# big: wih | whh | wfc | biasg | bfc
C_WIH, C_WHH, C_WFC, C_BG, C_BFC, C_BIG = 0, 2048, 4096, 4608, 5632, 5888


def _perm_ifog(a, axis=0):
    """[i,f,g,o] -> [i,g,f,o]: bank0 = (i,g) closes first and feeds the
    cell (sig_i, tanh_g, m1) while the PE still fills bank1 = (f,o)."""
    idx = np.concatenate([
        np.arange(0, H),          # i
        np.arange(2 * H, 3 * H),  # g
        np.arange(H, 2 * H),      # f
        np.arange(3 * H, 4 * H),  # o
    ])
    return np.take(a, idx, axis=axis)


def _build_bass():
    nc = bacc.Bacc("TRN2", target_bir_lowering=False, debug=False)

    x_ext = nc.declare_dram_parameter("x", [T, NG, R, F], BF16, isOutput=False)
    ws_ext = nc.declare_dram_parameter("ws", [128, C_SMALL], BF16, isOutput=False)
    wb_ext = nc.declare_dram_parameter("wb", [128, C_BIG], BF16, isOutput=False)
    out_ext = nc.declare_dram_parameter("out", [NG, R, H], F32, isOutput=True)

    with tile.TileContext(nc) as tc:
        with (
            tc.tile_pool(name="wpool", bufs=1) as wpool,
            tc.tile_pool(name="state", bufs=1) as state,
            tc.tile_pool(name="xpool", bufs=4) as xpool,
            tc.tile_pool(name="mixps", bufs=1, space="PSUM") as mixps,
            tc.tile_pool(name="sbx", bufs=4) as sbx,
            tc.tile_pool(name="sbh", bufs=2) as sbh,
            tc.tile_pool(name="gps", bufs=3, space="PSUM") as gps,
            tc.tile_pool(name="ew", bufs=2) as ew,
        ):
            # ---- weights: two packed DMAs (init-critical blob first) ----
            ws = wpool.tile([128, C_SMALL], BF16)
            nc.sync.dma_start(ws[:], ws_ext[:])
            wb = wpool.tile([128, C_BIG], BF16)
            nc.sync.dma_start(wb[:], wb_ext[:])

            bd = ws[0:96, C_BD:C_BD + 96]
            w1 = ws[:, C_W1:C_W2]
            w2 = ws[:, C_W2:C_ONES]
            ones = ws[0:1, C_ONES:C_B1]
            b1 = ws[0:1, C_B1:C_B2]
            b2 = ws[0:1, C_B2:C_SMALL]
            wih = wb[:, C_WIH:C_WHH]
            whh = wb[:, C_WHH:C_WFC]
            wfc = wb[:, C_WFC:C_BG]
            biasg = wb[0:1, C_BG:C_BFC]
            bfc = wb[0:1, C_BFC:C_BIG]

            # ---- x tiles: DMA the first 4 steps up-front ----
            xts = {}

            def stage_dma(s):
                xt = xpool.tile([R, F], BF16, tag="xt", name="xt")
                nc.sync.dma_start(xt[:], x_ext[s // NG, s % NG])
                xts[s] = xt

            for s in range(4):
                stage_dma(s)

            # PE warm-up: keep the PE busy through the weight-DMA window so
            # the clock gate opens and the pstate ramps before step 0.
            for wtag in ("mpx", "mph"):
                wu_ps = mixps.tile([128, 256], F32, tag=wtag, name=f"wu_{wtag}")
                for _ in range(30):
                    nc.tensor.matmul(wu_ps[0:R, 0:R], bd, bd,
                                     start=True, stop=True)

            # ---- persistent state ----
            hs = [state.tile([R, H], BF16, tag=f"h{g}", name=f"h{g}")
                  for g in range(NG)]
            cs = [state.tile([R, H], BF16, tag=f"c{g}", name=f"c{g}")
                  for g in range(NG)]

            msx = {}

            def stage_xmix(s):
                """x-mix for step s: PE mix matmuls + GPSIMD psum->sbuf cast."""
                xt = xts.pop(s)
                ps = mixps.tile([128, 256], F32, tag="mpx", name="mpx")
                for fc in range(2):
                    nc.tensor.matmul(
                        ps[:, fc * 128:fc * 128 + R],
                        xt[:, fc * 128:(fc + 1) * 128],
                        bd, start=True, stop=True)
                sb = sbx.tile([128, 256], BF16, tag="msx", name="msx")
                ps_v = ps[:].rearrange("p (c k) -> p c k", k=128)[:, :, 0:R]
                sb_v = sb[:].rearrange("p (c k) -> p c k", k=128)[:, :, 0:R]
                nc.vector.tensor_copy(sb_v, ps_v)
                msx[s] = sb

            def mix_h(hsb):
                """h-mix: PE matmuls + GPSIMD cast (on the recurrence path)."""
                ps = mixps.tile([128, 256], F32, tag="mph", name="mph")
                for fc in range(2):
                    nc.tensor.matmul(
                        ps[:, fc * 128:fc * 128 + R],
                        hsb[:, fc * 128:(fc + 1) * 128],
                        bd, start=True, stop=True)
                sb = sbh.tile([128, 256], BF16, tag="msh", name="msh")
                ps_v = ps[:].rearrange("p (c k) -> p c k", k=128)[:, :, 0:R]
                sb_v = sb[:].rearrange("p (c k) -> p c k", k=128)[:, :, 0:R]
                nc.vector.tensor_copy(sb_v, ps_v)
                return sb

            pending = {}

            def stage_bias(s):
                """open step s's gates psum with the bias rows."""
                ps = gps.tile([128, G4], F32, tag="gates", name="gates")
                for nch in range(2):
                    nc.tensor.matmul(ps[:, nch * 512:(nch + 1) * 512],
                                     ones, biasg[:, nch * 512:(nch + 1) * 512],
                                     start=True, stop=False)
                pending[s] = ps

            def stage_xg(s):
                """x-side matmuls into step s's gates psum."""
                gxT = msx.pop(s)
                ps = pending[s]
                for nch in range(2):
                    for fc in range(2):
                        nc.tensor.matmul(
                            ps[:, nch * 512:(nch + 1) * 512],
                            gxT[:, fc * 128:(fc + 1) * 128],
                            wih[:, fc * G4 + nch * 512:fc * G4 + (nch + 1) * 512],
                            start=False, stop=False)

            def close_gates(ps, ghT):
                """h-side matmuls; nch0 (psum bank0 = g,i) stops first."""
                for nch in range(2):
                    for fc in range(2):
                        nc.tensor.matmul(
                            ps[:, nch * 512:(nch + 1) * 512],
                            ghT[:, fc * 128:(fc + 1) * 128],
                            whh[:, fc * G4 + nch * 512:fc * G4 + (nch + 1) * 512],
                            start=False, stop=(fc == 1))

            hmul_pending = {}

            def cell(s):
                """Everything except the final h = sig_o * tanh(c), which is
                issued next iteration (flush_hmul) so the next h-mix cast is
                not stuck behind it in the DVE queue."""
                g = s % NG
                ps = pending.pop(s)
                # bank0 = (i,g): starts as soon as the first psum bank closes
                sig_i = ew.tile([R, H], BF16, tag="sigi", name="sig_i")
                nc.scalar.activation(sig_i[:], ps[0:R, 0:H],
                                     mybir.ActivationFunctionType.Sigmoid)
                tg = ew.tile([R, H], BF16, tag="tg", name="tg")
                nc.scalar.activation(tg[:], ps[0:R, H:2 * H],
                                     mybir.ActivationFunctionType.Tanh)
                m1 = ew.tile([R, H], BF16, tag="m1", name="m1")
                nc.vector.tensor_mul(m1[:], sig_i[:], tg[:])
                # bank1 = (f,o)
                sig_fo = ew.tile([R, 2 * H], BF16, tag="sigfo", name="sig_fo")
                nc.scalar.activation(sig_fo[:], ps[0:R, 2 * H:4 * H],
                                     mybir.ActivationFunctionType.Sigmoid)
                m2 = ew.tile([R, H], BF16, tag="m2", name="m2")
                nc.vector.tensor_mul(m2[:], sig_fo[:, 0:H], cs[g][:])
                nc.vector.tensor_add(cs[g][:], m1[:], m2[:])
                tc_t = ew.tile([R, H], BF16, tag="tc", name="tc")
                nc.scalar.activation(tc_t[:], cs[g][:],
                                     mybir.ActivationFunctionType.Tanh)
                nc.vector.tensor_mul(hs[g][:], sig_fo[:, H:2 * H], tc_t[:])

            def flush_hmul(s):
                pass

            def proj_init(gxT, w_t, bias_t, dst):
                """dst[96,256] = (gxT^T stacked) @ w + bias (init h0/c0)."""
                ps = gps.tile([128, G4], F32, tag="gates", name="gates")
                nc.tensor.matmul(ps[:, 0:H], ones, bias_t,
                                 start=True, stop=False)
                for fc in range(2):
                    nc.tensor.matmul(
                        ps[:, 0:H],
                        gxT[:, fc * 128:(fc + 1) * 128],
                        w_t[:, fc * H:(fc + 1) * H],
                        start=False, stop=(fc == 1))
                nc.vector.tensor_copy(dst[:], ps[0:R, 0:H])

            # ---- prologue: init h0/c0 (reuses step-0/1 x-mixes) ----
            stage_xmix(0)
            stage_xmix(1)
            for g in range(NG):
                proj_init(msx[g], w1, b1, hs[g])
                proj_init(msx[g], w2, b2, cs[g])
            stage_dma(4)
            stage_dma(5)
            stage_xmix(2)
            stage_xmix(3)
            stage_bias(0)
            stage_xg(0)

            # ---- recurrence ----
            for s in range(NSTEP):
                g = s % NG
                ghT = mix_h(hs[g])
                if s + 4 < NSTEP:
                    stage_xmix(s + 4)
                if s + 6 < NSTEP:
                    stage_dma(s + 6)
                if s + 1 < NSTEP:
                    stage_bias(s + 1)
                    stage_xg(s + 1)
                close_gates(pending[s], ghT)
                cell(s)

            # ---- final projection ----
            for g in range(NG):
                ghT = mix_h(hs[g])
                ps = gps.tile([128, G4], F32, tag="gates", name="gates")
                nc.tensor.matmul(ps[:, 0:H], ones, bfc, start=True, stop=False)
                for fc in range(2):
                    nc.tensor.matmul(
                        ps[:, 0:H],
                        ghT[:, fc * 128:(fc + 1) * 128],
                        wfc[:, fc * H:(fc + 1) * H],
                        start=False, stop=(fc == 1))
                o_sb = ew.tile([R, H], F32, tag="osb", name="osb")
                nc.scalar.activation(o_sb[:], ps[0:R, 0:H],
                                     mybir.ActivationFunctionType.Tanh)
                nc.sync.dma_start(out_ext[g], o_sb[:])

    nc.compile()
    return nc


_NC_CACHE = None


def kernel(x, G, W_ih, b_ih, W_hh, b_hh, W_h1, b_h1, W_h2, b_h2, W_fc, b_fc):
    global _NC_CACHE, LAST_EXEC_NS

    x = np.asarray(x)
    G = np.asarray(G, dtype=np.float32)

    # host-side staging
    # x: [B,T,N,F] -> per-core [T, NG, R, F] with b = core*B_LOC + g*BG + bb
    xs = np.asarray(x, dtype=np.float32).reshape(NCORES, NG, BG, T, N, F)
    xs = xs.transpose(0, 3, 1, 2, 4, 5).reshape(NCORES, T, NG, R, F)
    xs = xs.astype(ml_dtypes.bfloat16)

    bd = np.kron(np.eye(BG, dtype=np.float32), G.T)

    def _wt(w):  # [out, in] -> lhs-side [128, 2*out] (feat chunks along cols)
        wt = np.ascontiguousarray(np.asarray(w, np.float32).T)  # [in, out]
        return np.concatenate([wt[0:128], wt[128:256]], axis=1)

    ws = np.zeros((128, C_SMALL), np.float32)
    ws[0:96, C_BD:C_BD + 96] = bd
    ws[:, C_W1:C_W2] = _wt(W_h1)
    ws[:, C_W2:C_ONES] = _wt(W_h2)
    ws[0, C_ONES:C_B1] = 1.0
    ws[0, C_B1:C_B2] = np.asarray(b_h1, np.float32)
    ws[0, C_B2:C_SMALL] = np.asarray(b_h2, np.float32)

    wbig = np.zeros((128, C_BIG), np.float32)
    wbig[:, C_WIH:C_WHH] = _wt(_perm_ifog(np.asarray(W_ih)))
    wbig[:, C_WHH:C_WFC] = _wt(_perm_ifog(np.asarray(W_hh)))
    wbig[:, C_WFC:C_BG] = _wt(W_fc)
    wbig[0, C_BG:C_BFC] = _perm_ifog(
        np.asarray(b_ih, np.float32) + np.asarray(b_hh, np.float32))
    wbig[0, C_BFC:C_BIG] = np.asarray(b_fc, np.float32)

    ws = ws.astype(ml_dtypes.bfloat16)
    wbig = wbig.astype(ml_dtypes.bfloat16)

    if _NC_CACHE is None:
        _NC_CACHE = _build_bass()
    nc = _NC_CACHE

    in_maps = [dict(x=xs[core], ws=ws, wb=wbig) for core in range(NCORES)]

    res = run_bass_kernel_spmd(nc, in_maps, list(range(NCORES)), **RUN_KWARGS)
    LAST_EXEC_NS = res.exec_time_ns

    out = np.empty((B, N, H), np.float32)
    for core in range(NCORES):
        o = res.results[core]["out"].reshape(NG, BG, N, H)
        for g in range(NG):
            for bb in range(BG):
                out[core * B_LOC + g * BG + bb] = o[g, bb]
    return out


if __name__ == "__main__":
    rng = np.random.default_rng(0)
    ins = {
        "x": rng.standard_normal((B, T, N, F), np.float32),
        "G": rng.standard_normal((N, N), np.float32) / np.sqrt(N),
        "W_ih": rng.standard_normal((G4, F), np.float32) * 0.05,
        "b_ih": rng.standard_normal((G4,), np.float32) * 0.05,
        "W_hh": rng.standard_normal((G4, H), np.float32) * 0.05,
        "b_hh": rng.standard_normal((G4,), np.float32) * 0.05,
        "W_h1": rng.standard_normal((H, F), np.float32) * 0.05,
        "b_h1": rng.standard_normal((H,), np.float32) * 0.05,
        "W_h2": rng.standard_normal((H, F), np.float32) * 0.05,
        "b_h2": rng.standard_normal((H,), np.float32) * 0.05,
        "W_fc": rng.standard_normal((H, H), np.float32) * 0.05,
        "b_fc": rng.standard_normal((H,), np.float32) * 0.05,
    }
    out = kernel(**ins)
    print("out", out.shape, out.dtype, float(np.abs(out).mean()))


# revision 20
# speedup vs baseline: 1.1830x; 1.0021x over previous
"""Graph-LSTM encoder kernel for 8x Trainium2 NeuronCores.

Problem: B,T,N,F,H = 64,50,24,256,256
    h = graph_linear(G, x0, W_h1, b_h1); c = graph_linear(G, x0, W_h2, b_h2)
    per t: gates = GL(G, x_t, W_ih, b_ih) + GL(G, h, W_hh, b_hh)  (LSTM cell)
    out = tanh(GL(G, h_T, W_fc, b_fc))
where GL(G, x, W, b) = einsum('nm,bmf->bnf', G, x @ W.T) + b
                     = (G . x) @ W.T + b      (mix commutes with projection)

Sharding: data-parallel over batch, 8 batches/core. Per core, batches are
split into 2 groups of 4 (96 rows of 24 nodes each) which pipeline against
each other (PE on one group while ACT/DVE handle the other).

Layouts per group (rows = (batch-in-group, node) on partitions):
  state   h [96,256] bf16, c [96,256] f32 (persistent SBUF)
  mix     mm(lhsT=z[96, fc*128:+128], rhs=BD[96,96]) -> psum[128, fc*128:+96]
          where BD = kron(I4, G^T): block-diag node mix, contracted over rows.
  gates   [128,1024] psum (rows 96+ garbage), accumulated as
            ones[1,128]^T @ bias[1,512]        (bias row, start=True)
          + GzT blocks ^T @ W blocks           (bf16, M=128 for fast LDW)
  cell    gates pre-permuted to [g,i | f,o]: one tanh(g) (can start when
          psum bank0 closes), ONE merged sigmoid over [i,f,o] (768 cols),
          then bf16 DVE cell math. All mix-psum casts run on the idle
          GPSIMD engine so the DVE only does cell arithmetic.

Software pipeline (per loop iteration s, group g = s%2):
  PE:     h-mix(s) | bias+x-gates(s+2) | x-mix(s+4) | h-gates(s)
  GPSIMD: h-mix cast(s), x-mix cast(s+4)
  Sync:   x DMA(s+6)
  ACT/DVE: cell(s)
Each stage consumes data produced >= 2 iterations earlier, so no
intra-iteration cross-engine stalls on the PE queue.

Weights arrive in 2 packed DMAs (init-critical small blob first), not 15.
"""

import sys

sys.path.insert(0, "/opt/trn_rl_repo")

import numpy as np
import ml_dtypes

import concourse.bacc as bacc
import concourse.mybir as mybir
import concourse.tile as tile
from concourse.bass_utils import run_bass_kernel_spmd

B, T, N, F, H = 64, 50, 24, 256, 256
NCORES = 8
B_LOC = B // NCORES      # 8 batches per core
NG = 2                   # pipeline groups per core
BG = B_LOC // NG         # 4 batches per group
R = BG * N               # 96 rows per group
G4 = 4 * H               # 1024 gate width
NSTEP = T * NG

F32 = mybir.dt.float32
BF16 = mybir.dt.bfloat16

LAST_EXEC_NS = None
RUN_KWARGS = {}

# --- packed weight blob column offsets (bf16, [128, *]) ---
# small (init-critical): bd | w12 (w1/w2 interleaved per fc) | ones | b12
C_BD, C_W12, C_ONES, C_B12, C_SMALL = 0, 96, 1120, 1248, 1760
# big: wih | whh | wfc | biasg | bfc
C_WIH, C_WHH, C_WFC, C_BG, C_BFC, C_BIG = 0, 2048, 4096, 4608, 5632, 5888


def _perm_ifog(a, axis=0):
    """[i,f,g,o] -> [i,g,f,o]: bank0 = (i,g) closes first and feeds the
    cell (sig_i, tanh_g, m1) while the PE still fills bank1 = (f,o)."""
    idx = np.concatenate([
        np.arange(0, H),          # i
        np.arange(2 * H, 3 * H),  # g
        np.arange(H, 2 * H),      # f
        np.arange(3 * H, 4 * H),  # o
    ])
    return np.take(a, idx, axis=axis)


def _build_bass():
    nc = bacc.Bacc("TRN2", target_bir_lowering=False, debug=False)

    x_ext = nc.declare_dram_parameter("x", [T, NG, R, F], BF16, isOutput=False)
    ws_ext = nc.declare_dram_parameter("ws", [128, C_SMALL], BF16, isOutput=False)
    wb_ext = nc.declare_dram_parameter("wb", [128, C_BIG], BF16, isOutput=False)
    out_ext = nc.declare_dram_parameter("out", [NG, R, H], F32, isOutput=True)

    with tile.TileContext(nc) as tc:
        with (
            tc.tile_pool(name="wpool", bufs=1) as wpool,
            tc.tile_pool(name="state", bufs=1) as state,
            tc.tile_pool(name="xpool", bufs=4) as xpool,
            tc.tile_pool(name="mixps", bufs=1, space="PSUM") as mixps,
            tc.tile_pool(name="sbx", bufs=4) as sbx,
            tc.tile_pool(name="sbh", bufs=2) as sbh,
            tc.tile_pool(name="gps", bufs=3, space="PSUM") as gps,
            tc.tile_pool(name="ew", bufs=2) as ew,
        ):
            # ---- weights + first x tiles: init-critical DMAs first ----
            ws = wpool.tile([128, C_SMALL], BF16)
            nc.sync.dma_start(ws[:], ws_ext[:])

            xts = {}

            def stage_dma(s):
                xt = xpool.tile([R, F], BF16, tag="xt", name="xt")
                nc.sync.dma_start(xt[:], x_ext[s // NG, s % NG])
                xts[s] = xt

            stage_dma(0)
            stage_dma(1)
            wb = wpool.tile([128, C_BIG], BF16)
            nc.sync.dma_start(wb[:], wb_ext[:])
            stage_dma(2)
            stage_dma(3)

            bd = ws[0:96, C_BD:C_BD + 96]
            w12 = ws[:, C_W12:C_ONES]
            ones = ws[0:1, C_ONES:C_B12]
            b12 = ws[0:1, C_B12:C_SMALL]
            wih = wb[:, C_WIH:C_WHH]
            whh = wb[:, C_WHH:C_WFC]
            wfc = wb[:, C_WFC:C_BG]
            biasg = wb[0:1, C_BG:C_BFC]
            bfc = wb[0:1, C_BFC:C_BIG]

            # ---- persistent state ----
            hs = [state.tile([R, H], BF16, tag=f"h{g}", name=f"h{g}")
                  for g in range(NG)]
            cs = [state.tile([R, H], BF16, tag=f"c{g}", name=f"c{g}")
                  for g in range(NG)]

            msx = {}

            def stage_xmix(s):
                """x-mix for step s: PE mix matmuls + GPSIMD psum->sbuf cast."""
                xt = xts.pop(s)
                ps = mixps.tile([128, 256], F32, tag="mpx", name="mpx")
                for fc in range(2):
                    nc.tensor.matmul(
                        ps[:, fc * 128:fc * 128 + R],
                        xt[:, fc * 128:(fc + 1) * 128],
                        bd, start=True, stop=True)
                sb = sbx.tile([128, 256], BF16, tag="msx", name="msx")
                ps_v = ps[:].rearrange("p (c k) -> p c k", k=128)[:, :, 0:R]
                sb_v = sb[:].rearrange("p (c k) -> p c k", k=128)[:, :, 0:R]
                nc.vector.tensor_copy(sb_v, ps_v)
                msx[s] = sb

            def mix_h(hsb):
                """h-mix: PE matmuls + GPSIMD cast (on the recurrence path)."""
                ps = mixps.tile([128, 256], F32, tag="mph", name="mph")
                for fc in range(2):
                    nc.tensor.matmul(
                        ps[:, fc * 128:fc * 128 + R],
                        hsb[:, fc * 128:(fc + 1) * 128],
                        bd, start=True, stop=True)
                sb = sbh.tile([128, 256], BF16, tag="msh", name="msh")
                ps_v = ps[:].rearrange("p (c k) -> p c k", k=128)[:, :, 0:R]
                sb_v = sb[:].rearrange("p (c k) -> p c k", k=128)[:, :, 0:R]
                nc.vector.tensor_copy(sb_v, ps_v)
                return sb

            pending = {}

            def stage_bias(s):
                """open step s's gates psum with the bias rows."""
                ps = gps.tile([128, G4], F32, tag="gates", name="gates")
                for nch in range(2):
                    nc.tensor.matmul(ps[:, nch * 512:(nch + 1) * 512],
                                     ones, biasg[:, nch * 512:(nch + 1) * 512],
                                     start=True, stop=False)
                pending[s] = ps

            def stage_xg(s):
                """x-side matmuls into step s's gates psum."""
                gxT = msx.pop(s)
                ps = pending[s]
                for nch in range(2):
                    for fc in range(2):
                        nc.tensor.matmul(
                            ps[:, nch * 512:(nch + 1) * 512],
                            gxT[:, fc * 128:(fc + 1) * 128],
                            wih[:, fc * G4 + nch * 512:fc * G4 + (nch + 1) * 512],
                            start=False, stop=False)

            def close_gates(ps, ghT):
                """h-side matmuls; nch0 (psum bank0 = g,i) stops first."""
                for nch in range(2):
                    for fc in range(2):
                        nc.tensor.matmul(
                            ps[:, nch * 512:(nch + 1) * 512],
                            ghT[:, fc * 128:(fc + 1) * 128],
                            whh[:, fc * G4 + nch * 512:fc * G4 + (nch + 1) * 512],
                            start=False, stop=(fc == 1))

            hmul_pending = {}

            def cell(s):
                """Everything except the final h = sig_o * tanh(c), which is
                issued next iteration (flush_hmul) so the next h-mix cast is
                not stuck behind it in the DVE queue."""
                g = s % NG
                ps = pending.pop(s)
                # bank0 = (i,g): starts as soon as the first psum bank closes
                sig_i = ew.tile([R, H], BF16, tag="sigi", name="sig_i")
                nc.scalar.activation(sig_i[:], ps[0:R, 0:H],
                                     mybir.ActivationFunctionType.Sigmoid)
                tg = ew.tile([R, H], BF16, tag="tg", name="tg")
                nc.scalar.activation(tg[:], ps[0:R, H:2 * H],
                                     mybir.ActivationFunctionType.Tanh)
                m1 = ew.tile([R, H], BF16, tag="m1", name="m1")
                nc.vector.tensor_mul(m1[:], sig_i[:], tg[:])
                # bank1 = (f,o)
                sig_fo = ew.tile([R, 2 * H], BF16, tag="sigfo", name="sig_fo")
                nc.scalar.activation(sig_fo[:], ps[0:R, 2 * H:4 * H],
                                     mybir.ActivationFunctionType.Sigmoid)
                m2 = ew.tile([R, H], BF16, tag="m2", name="m2")
                nc.vector.tensor_mul(m2[:], sig_fo[:, 0:H], cs[g][:])
                nc.vector.tensor_add(cs[g][:], m1[:], m2[:])
                tc_t = ew.tile([R, H], BF16, tag="tc", name="tc")
                nc.scalar.activation(tc_t[:], cs[g][:],
                                     mybir.ActivationFunctionType.Tanh)
                nc.vector.tensor_mul(hs[g][:], sig_fo[:, H:2 * H], tc_t[:])

            def flush_hmul(s):
                pass

            def proj_init(gxT, g):
                """h0/c0 for group g in one psum pass: [w1|w2] packed."""
                ps = gps.tile([128, G4], F32, tag="gates", name="gates")
                nc.tensor.matmul(ps[:, 0:2 * H], ones, b12,
                                 start=True, stop=False)
                for fc in range(2):
                    nc.tensor.matmul(
                        ps[:, 0:2 * H],
                        gxT[:, fc * 128:(fc + 1) * 128],
                        w12[:, fc * 2 * H:(fc + 1) * 2 * H],
                        start=False, stop=(fc == 1))
                nc.vector.tensor_copy(hs[g][:], ps[0:R, 0:H])
                nc.vector.tensor_copy(cs[g][:], ps[0:R, H:2 * H])

            # ---- prologue ----
            # x-mix(0) first, then the PE warm-up (psum from the gates pool)
            # runs back-to-back behind it: the PE stays busy through the
            # big-weight DMA window and the pstate ramp finishes before init.
            stage_xmix(0)
            for i in range(2):
                wu_ps = gps.tile([128, G4], F32, tag="gates", name="wu")
                for _ in range(25):
                    nc.tensor.matmul(wu_ps[0:R, 0:R], bd, bd,
                                     start=True, stop=True)
            stage_xmix(1)
            stage_xmix(2)
            stage_xmix(3)
            # init h0/c0 (reuses the step-0/1 x-mixes; w1/w2 in one pass)
            for g in range(NG):
                proj_init(msx[g], g)
            stage_dma(4)
            stage_dma(5)
            stage_bias(0)
            stage_xg(0)

            # ---- recurrence ----
            for s in range(NSTEP):
                g = s % NG
                ghT = mix_h(hs[g])
                if s + 4 < NSTEP:
                    stage_xmix(s + 4)
                if s + 6 < NSTEP:
                    stage_dma(s + 6)
                if s + 1 < NSTEP:
                    stage_bias(s + 1)
                    stage_xg(s + 1)
                close_gates(pending[s], ghT)
                cell(s)

            # ---- final projection ----
            for g in range(NG):
                ghT = mix_h(hs[g])
                ps = gps.tile([128, G4], F32, tag="gates", name="gates")
                nc.tensor.matmul(ps[:, 0:H], ones, bfc, start=True, stop=False)
                for fc in range(2):
                    nc.tensor.matmul(
                        ps[:, 0:H],
                        ghT[:, fc * 128:(fc + 1) * 128],
                        wfc[:, fc * H:(fc + 1) * H],
                        start=False, stop=(fc == 1))
                o_sb = ew.tile([R, H], F32, tag="osb", name="osb")
                nc.scalar.activation(o_sb[:], ps[0:R, 0:H],
                                     mybir.ActivationFunctionType.Tanh)
                nc.sync.dma_start(out_ext[g], o_sb[:])

    nc.compile()
    return nc


_NC_CACHE = None


def kernel(x, G, W_ih, b_ih, W_hh, b_hh, W_h1, b_h1, W_h2, b_h2, W_fc, b_fc):
    global _NC_CACHE, LAST_EXEC_NS

    x = np.asarray(x)
    G = np.asarray(G, dtype=np.float32)

    # host-side staging
    # x: [B,T,N,F] -> per-core [T, NG, R, F] with b = core*B_LOC + g*BG + bb
    xs = np.asarray(x, dtype=np.float32).reshape(NCORES, NG, BG, T, N, F)
    xs = xs.transpose(0, 3, 1, 2, 4, 5).reshape(NCORES, T, NG, R, F)
    xs = xs.astype(ml_dtypes.bfloat16)

    bd = np.kron(np.eye(BG, dtype=np.float32), G.T)

    def _wt(w):  # [out, in] -> lhs-side [128, 2*out] (feat chunks along cols)
        wt = np.ascontiguousarray(np.asarray(w, np.float32).T)  # [in, out]
        return np.concatenate([wt[0:128], wt[128:256]], axis=1)

    ws = np.zeros((128, C_SMALL), np.float32)
    ws[0:96, C_BD:C_BD + 96] = bd
    w1t, w2t = _wt(W_h1), _wt(W_h2)  # [128, 2H] each, fc chunks along cols
    for fc in range(2):
        ws[:, C_W12 + fc * 512:C_W12 + fc * 512 + 256] = w1t[:, fc * 256:(fc + 1) * 256]
        ws[:, C_W12 + fc * 512 + 256:C_W12 + (fc + 1) * 512] = w2t[:, fc * 256:(fc + 1) * 256]
    ws[0, C_ONES:C_B12] = 1.0
    ws[0, C_B12:C_B12 + 256] = np.asarray(b_h1, np.float32)
    ws[0, C_B12 + 256:C_SMALL] = np.asarray(b_h2, np.float32)

    wbig = np.zeros((128, C_BIG), np.float32)
    wbig[:, C_WIH:C_WHH] = _wt(_perm_ifog(np.asarray(W_ih)))
    wbig[:, C_WHH:C_WFC] = _wt(_perm_ifog(np.asarray(W_hh)))
    wbig[:, C_WFC:C_BG] = _wt(W_fc)
    wbig[0, C_BG:C_BFC] = _perm_ifog(
        np.asarray(b_ih, np.float32) + np.asarray(b_hh, np.float32))
    wbig[0, C_BFC:C_BIG] = np.asarray(b_fc, np.float32)

    ws = ws.astype(ml_dtypes.bfloat16)
    wbig = wbig.astype(ml_dtypes.bfloat16)

    if _NC_CACHE is None:
        _NC_CACHE = _build_bass()
    nc = _NC_CACHE

    in_maps = [dict(x=xs[core], ws=ws, wb=wbig) for core in range(NCORES)]

    res = run_bass_kernel_spmd(nc, in_maps, list(range(NCORES)), **RUN_KWARGS)
    LAST_EXEC_NS = res.exec_time_ns

    out = np.empty((B, N, H), np.float32)
    for core in range(NCORES):
        o = res.results[core]["out"].reshape(NG, BG, N, H)
        for g in range(NG):
            for bb in range(BG):
                out[core * B_LOC + g * BG + bb] = o[g, bb]
    return out


if __name__ == "__main__":
    rng = np.random.default_rng(0)
    ins = {
        "x": rng.standard_normal((B, T, N, F), np.float32),
        "G": rng.standard_normal((N, N), np.float32) / np.sqrt(N),
        "W_ih": rng.standard_normal((G4, F), np.float32) * 0.05,
        "b_ih": rng.standard_normal((G4,), np.float32) * 0.05,
        "W_hh": rng.standard_normal((G4, H), np.float32) * 0.05,
        "b_hh": rng.standard_normal((G4,), np.float32) * 0.05,
        "W_h1": rng.standard_normal((H, F), np.float32) * 0.05,
        "b_h1": rng.standard_normal((H,), np.float32) * 0.05,
        "W_h2": rng.standard_normal((H, F), np.float32) * 0.05,
        "b_h2": rng.standard_normal((H,), np.float32) * 0.05,
        "W_fc": rng.standard_normal((H, H), np.float32) * 0.05,
        "b_fc": rng.standard_normal((H,), np.float32) * 0.05,
    }
    out = kernel(**ins)
    print("out", out.shape, out.dtype, float(np.abs(out).mean()))


# revision 21
# speedup vs baseline: 1.1855x; 1.0021x over previous
"""Graph-LSTM encoder kernel for 8x Trainium2 NeuronCores.

Problem: B,T,N,F,H = 64,50,24,256,256
    h = graph_linear(G, x0, W_h1, b_h1); c = graph_linear(G, x0, W_h2, b_h2)
    per t: gates = GL(G, x_t, W_ih, b_ih) + GL(G, h, W_hh, b_hh)  (LSTM cell)
    out = tanh(GL(G, h_T, W_fc, b_fc))
where GL(G, x, W, b) = einsum('nm,bmf->bnf', G, x @ W.T) + b
                     = (G . x) @ W.T + b      (mix commutes with projection)

Sharding: data-parallel over batch, 8 batches/core. Per core, batches are
split into 2 groups of 4 (96 rows of 24 nodes each) which pipeline against
each other (PE on one group while ACT/DVE handle the other).

Layouts per group (rows = (batch-in-group, node) on partitions):
  state   h [96,256] bf16, c [96,256] f32 (persistent SBUF)
  mix     mm(lhsT=z[96, fc*128:+128], rhs=BD[96,96]) -> psum[128, fc*128:+96]
          where BD = kron(I4, G^T): block-diag node mix, contracted over rows.
  gates   [128,1024] psum (rows 96+ garbage), accumulated as
            ones[1,128]^T @ bias[1,512]        (bias row, start=True)
          + GzT blocks ^T @ W blocks           (bf16, M=128 for fast LDW)
  cell    gates pre-permuted to [g,i | f,o]: one tanh(g) (can start when
          psum bank0 closes), ONE merged sigmoid over [i,f,o] (768 cols),
          then bf16 DVE cell math. All mix-psum casts run on the idle
          GPSIMD engine so the DVE only does cell arithmetic.

Software pipeline (per loop iteration s, group g = s%2):
  PE:     h-mix(s) | bias+x-gates(s+2) | x-mix(s+4) | h-gates(s)
  GPSIMD: h-mix cast(s), x-mix cast(s+4)
  Sync:   x DMA(s+6)
  ACT/DVE: cell(s)
Each stage consumes data produced >= 2 iterations earlier, so no
intra-iteration cross-engine stalls on the PE queue.

Weights arrive in 2 packed DMAs (init-critical small blob first), not 15.
"""

import sys

sys.path.insert(0, "/opt/trn_rl_repo")

import numpy as np
import ml_dtypes

import concourse.bacc as bacc
import concourse.mybir as mybir
import concourse.tile as tile
from concourse.bass_utils import run_bass_kernel_spmd

B, T, N, F, H = 64, 50, 24, 256, 256
NCORES = 8
B_LOC = B // NCORES      # 8 batches per core
NG = 2                   # pipeline groups per core
BG = B_LOC // NG         # 4 batches per group
R = BG * N               # 96 rows per group
G4 = 4 * H               # 1024 gate width
NSTEP = T * NG

F32 = mybir.dt.float32
BF16 = mybir.dt.bfloat16

LAST_EXEC_NS = None
RUN_KWARGS = {}

# --- packed weight blob column offsets (bf16, [128, *]) ---
# small (init-critical): bd | w12 (w1/w2 interleaved per fc) | ones | b12
C_BD, C_W12, C_ONES, C_B12, C_SMALL = 0, 96, 1120, 1248, 1760
# big: wih | whh | wfc | biasg | bfc
C_WIH, C_WHH, C_WFC, C_BG, C_BFC, C_BIG = 0, 2048, 4096, 4608, 5632, 5888


def _perm_ifog(a, axis=0):
    """[i,f,g,o] -> [i,g,f,o]: bank0 = (i,g) closes first and feeds the
    cell (sig_i, tanh_g, m1) while the PE still fills bank1 = (f,o)."""
    idx = np.concatenate([
        np.arange(0, H),          # i
        np.arange(2 * H, 3 * H),  # g
        np.arange(H, 2 * H),      # f
        np.arange(3 * H, 4 * H),  # o
    ])
    return np.take(a, idx, axis=axis)


def _build_bass():
    nc = bacc.Bacc("TRN2", target_bir_lowering=False, debug=False)

    x_ext = nc.declare_dram_parameter("x", [T, NG, R, F], BF16, isOutput=False)
    ws_ext = nc.declare_dram_parameter("ws", [128, C_SMALL], BF16, isOutput=False)
    wb_ext = nc.declare_dram_parameter("wb", [128, C_BIG], BF16, isOutput=False)
    out_ext = nc.declare_dram_parameter("out", [NG, R, H], F32, isOutput=True)

    with tile.TileContext(nc) as tc:
        with (
            tc.tile_pool(name="wpool", bufs=1) as wpool,
            tc.tile_pool(name="state", bufs=1) as state,
            tc.tile_pool(name="xpool", bufs=4) as xpool,
            tc.tile_pool(name="mixps", bufs=1, space="PSUM") as mixps,
            tc.tile_pool(name="sbx", bufs=4) as sbx,
            tc.tile_pool(name="sbh", bufs=2) as sbh,
            tc.tile_pool(name="gps", bufs=3, space="PSUM") as gps,
            tc.tile_pool(name="ew", bufs=2) as ew,
        ):
            # ---- weights + first x tiles: init-critical DMAs first ----
            ws = wpool.tile([128, C_SMALL], BF16)
            nc.sync.dma_start(ws[:], ws_ext[:])

            xts = {}

            def stage_dma(s):
                xt = xpool.tile([R, F], BF16, tag="xt", name="xt")
                nc.sync.dma_start(xt[:], x_ext[s // NG, s % NG])
                xts[s] = xt

            stage_dma(0)
            stage_dma(1)
            wb = wpool.tile([128, C_BIG], BF16)
            nc.sync.dma_start(wb[:], wb_ext[:])
            stage_dma(2)
            stage_dma(3)

            bd = ws[0:96, C_BD:C_BD + 96]
            w12 = ws[:, C_W12:C_ONES]
            ones = ws[0:1, C_ONES:C_B12]
            b12 = ws[0:1, C_B12:C_SMALL]
            wih = wb[:, C_WIH:C_WHH]
            whh = wb[:, C_WHH:C_WFC]
            wfc = wb[:, C_WFC:C_BG]
            biasg = wb[0:1, C_BG:C_BFC]
            bfc = wb[0:1, C_BFC:C_BIG]

            # ---- persistent state ----
            hs = [state.tile([R, H], BF16, tag=f"h{g}", name=f"h{g}")
                  for g in range(NG)]
            cs = [state.tile([R, H], BF16, tag=f"c{g}", name=f"c{g}")
                  for g in range(NG)]

            msx = {}

            def stage_xmix(s):
                """x-mix for step s: PE mix matmuls + GPSIMD psum->sbuf cast."""
                xt = xts.pop(s)
                ps = mixps.tile([128, 256], F32, tag="mpx", name="mpx")
                for fc in range(2):
                    nc.tensor.matmul(
                        ps[:, fc * 128:fc * 128 + R],
                        xt[:, fc * 128:(fc + 1) * 128],
                        bd, start=True, stop=True)
                sb = sbx.tile([128, 256], BF16, tag="msx", name="msx")
                ps_v = ps[:].rearrange("p (c k) -> p c k", k=128)[:, :, 0:R]
                sb_v = sb[:].rearrange("p (c k) -> p c k", k=128)[:, :, 0:R]
                nc.vector.tensor_copy(sb_v, ps_v)
                msx[s] = sb

            def mix_h(hsb):
                """h-mix: PE matmuls + GPSIMD cast (on the recurrence path)."""
                ps = mixps.tile([128, 256], F32, tag="mph", name="mph")
                for fc in range(2):
                    nc.tensor.matmul(
                        ps[:, fc * 128:fc * 128 + R],
                        hsb[:, fc * 128:(fc + 1) * 128],
                        bd, start=True, stop=True)
                sb = sbh.tile([128, 256], BF16, tag="msh", name="msh")
                ps_v = ps[:].rearrange("p (c k) -> p c k", k=128)[:, :, 0:R]
                sb_v = sb[:].rearrange("p (c k) -> p c k", k=128)[:, :, 0:R]
                nc.vector.tensor_copy(sb_v, ps_v)
                return sb

            pending = {}

            def stage_bias(s):
                """open step s's gates psum with the bias rows."""
                ps = gps.tile([128, G4], F32, tag="gates", name="gates")
                for nch in range(2):
                    nc.tensor.matmul(ps[:, nch * 512:(nch + 1) * 512],
                                     ones, biasg[:, nch * 512:(nch + 1) * 512],
                                     start=True, stop=False)
                pending[s] = ps

            def stage_xg(s):
                """x-side matmuls into step s's gates psum."""
                gxT = msx.pop(s)
                ps = pending[s]
                for nch in range(2):
                    for fc in range(2):
                        nc.tensor.matmul(
                            ps[:, nch * 512:(nch + 1) * 512],
                            gxT[:, fc * 128:(fc + 1) * 128],
                            wih[:, fc * G4 + nch * 512:fc * G4 + (nch + 1) * 512],
                            start=False, stop=False)

            def close_gates(ps, ghT):
                """h-side matmuls; nch0 (psum bank0 = g,i) stops first."""
                for nch in range(2):
                    for fc in range(2):
                        nc.tensor.matmul(
                            ps[:, nch * 512:(nch + 1) * 512],
                            ghT[:, fc * 128:(fc + 1) * 128],
                            whh[:, fc * G4 + nch * 512:fc * G4 + (nch + 1) * 512],
                            start=False, stop=(fc == 1))

            hmul_pending = {}

            def cell(s):
                """Everything except the final h = sig_o * tanh(c), which is
                issued next iteration (flush_hmul) so the next h-mix cast is
                not stuck behind it in the DVE queue."""
                g = s % NG
                ps = pending.pop(s)
                # bank0 = (i,g): starts as soon as the first psum bank closes
                sig_i = ew.tile([R, H], BF16, tag="sigi", name="sig_i")
                nc.scalar.activation(sig_i[:], ps[0:R, 0:H],
                                     mybir.ActivationFunctionType.Sigmoid)
                tg = ew.tile([R, H], BF16, tag="tg", name="tg")
                nc.scalar.activation(tg[:], ps[0:R, H:2 * H],
                                     mybir.ActivationFunctionType.Tanh)
                m1 = ew.tile([R, H], BF16, tag="m1", name="m1")
                nc.vector.tensor_mul(m1[:], sig_i[:], tg[:])
                # bank1 = (f,o)
                sig_fo = ew.tile([R, 2 * H], BF16, tag="sigfo", name="sig_fo")
                nc.scalar.activation(sig_fo[:], ps[0:R, 2 * H:4 * H],
                                     mybir.ActivationFunctionType.Sigmoid)
                m2 = ew.tile([R, H], BF16, tag="m2", name="m2")
                nc.vector.tensor_mul(m2[:], sig_fo[:, 0:H], cs[g][:])
                nc.vector.tensor_add(cs[g][:], m1[:], m2[:])
                tc_t = ew.tile([R, H], BF16, tag="tc", name="tc")
                nc.scalar.activation(tc_t[:], cs[g][:],
                                     mybir.ActivationFunctionType.Tanh)
                nc.vector.tensor_mul(hs[g][:], sig_fo[:, H:2 * H], tc_t[:])

            def flush_hmul(s):
                pass

            def proj_init(gxT, g):
                """h0/c0 for group g in one psum pass: [w1|w2] packed."""
                ps = gps.tile([128, G4], F32, tag="gates", name="gates")
                nc.tensor.matmul(ps[:, 0:2 * H], ones, b12,
                                 start=True, stop=False)
                for fc in range(2):
                    nc.tensor.matmul(
                        ps[:, 0:2 * H],
                        gxT[:, fc * 128:(fc + 1) * 128],
                        w12[:, fc * 2 * H:(fc + 1) * 2 * H],
                        start=False, stop=(fc == 1))
                nc.vector.tensor_copy(hs[g][:], ps[0:R, 0:H])
                nc.vector.tensor_copy(cs[g][:], ps[0:R, H:2 * H])

            # ---- prologue ----
            # x-mix(0) first, then the PE warm-up (psum from the gates pool)
            # runs back-to-back behind it: the PE stays busy through the
            # big-weight DMA window and the pstate ramp finishes before init.
            stage_xmix(0)
            for i in range(2):
                wu_ps = gps.tile([128, G4], F32, tag="gates", name="wu")
                for _ in range(12):
                    nc.tensor.matmul(wu_ps[0:R, 0:R], bd, bd,
                                     start=True, stop=True)
            stage_xmix(1)
            stage_xmix(2)
            stage_xmix(3)
            # init h0/c0 (reuses the step-0/1 x-mixes; w1/w2 in one pass)
            for g in range(NG):
                proj_init(msx[g], g)
            stage_dma(4)
            stage_dma(5)
            stage_bias(0)
            stage_xg(0)

            # ---- recurrence ----
            for s in range(NSTEP):
                g = s % NG
                ghT = mix_h(hs[g])
                if s + 4 < NSTEP:
                    stage_xmix(s + 4)
                if s + 6 < NSTEP:
                    stage_dma(s + 6)
                if s + 1 < NSTEP:
                    stage_bias(s + 1)
                    stage_xg(s + 1)
                close_gates(pending[s], ghT)
                cell(s)

            # ---- final projection ----
            for g in range(NG):
                ghT = mix_h(hs[g])
                ps = gps.tile([128, G4], F32, tag="gates", name="gates")
                nc.tensor.matmul(ps[:, 0:H], ones, bfc, start=True, stop=False)
                for fc in range(2):
                    nc.tensor.matmul(
                        ps[:, 0:H],
                        ghT[:, fc * 128:(fc + 1) * 128],
                        wfc[:, fc * H:(fc + 1) * H],
                        start=False, stop=(fc == 1))
                o_sb = ew.tile([R, H], F32, tag="osb", name="osb")
                nc.scalar.activation(o_sb[:], ps[0:R, 0:H],
                                     mybir.ActivationFunctionType.Tanh)
                nc.sync.dma_start(out_ext[g], o_sb[:])

    nc.compile()
    return nc


_NC_CACHE = None


def kernel(x, G, W_ih, b_ih, W_hh, b_hh, W_h1, b_h1, W_h2, b_h2, W_fc, b_fc):
    global _NC_CACHE, LAST_EXEC_NS

    x = np.asarray(x)
    G = np.asarray(G, dtype=np.float32)

    # host-side staging
    # x: [B,T,N,F] -> per-core [T, NG, R, F] with b = core*B_LOC + g*BG + bb
    xs = np.asarray(x, dtype=np.float32).reshape(NCORES, NG, BG, T, N, F)
    xs = xs.transpose(0, 3, 1, 2, 4, 5).reshape(NCORES, T, NG, R, F)
    xs = xs.astype(ml_dtypes.bfloat16)

    bd = np.kron(np.eye(BG, dtype=np.float32), G.T)

    def _wt(w):  # [out, in] -> lhs-side [128, 2*out] (feat chunks along cols)
        wt = np.ascontiguousarray(np.asarray(w, np.float32).T)  # [in, out]
        return np.concatenate([wt[0:128], wt[128:256]], axis=1)

    ws = np.zeros((128, C_SMALL), np.float32)
    ws[0:96, C_BD:C_BD + 96] = bd
    w1t, w2t = _wt(W_h1), _wt(W_h2)  # [128, 2H] each, fc chunks along cols
    for fc in range(2):
        ws[:, C_W12 + fc * 512:C_W12 + fc * 512 + 256] = w1t[:, fc * 256:(fc + 1) * 256]
        ws[:, C_W12 + fc * 512 + 256:C_W12 + (fc + 1) * 512] = w2t[:, fc * 256:(fc + 1) * 256]
    ws[0, C_ONES:C_B12] = 1.0
    ws[0, C_B12:C_B12 + 256] = np.asarray(b_h1, np.float32)
    ws[0, C_B12 + 256:C_SMALL] = np.asarray(b_h2, np.float32)

    wbig = np.zeros((128, C_BIG), np.float32)
    wbig[:, C_WIH:C_WHH] = _wt(_perm_ifog(np.asarray(W_ih)))
    wbig[:, C_WHH:C_WFC] = _wt(_perm_ifog(np.asarray(W_hh)))
    wbig[:, C_WFC:C_BG] = _wt(W_fc)
    wbig[0, C_BG:C_BFC] = _perm_ifog(
        np.asarray(b_ih, np.float32) + np.asarray(b_hh, np.float32))
    wbig[0, C_BFC:C_BIG] = np.asarray(b_fc, np.float32)

    ws = ws.astype(ml_dtypes.bfloat16)
    wbig = wbig.astype(ml_dtypes.bfloat16)

    if _NC_CACHE is None:
        _NC_CACHE = _build_bass()
    nc = _NC_CACHE

    in_maps = [dict(x=xs[core], ws=ws, wb=wbig) for core in range(NCORES)]

    res = run_bass_kernel_spmd(nc, in_maps, list(range(NCORES)), **RUN_KWARGS)
    LAST_EXEC_NS = res.exec_time_ns

    out = np.empty((B, N, H), np.float32)
    for core in range(NCORES):
        o = res.results[core]["out"].reshape(NG, BG, N, H)
        for g in range(NG):
            for bb in range(BG):
                out[core * B_LOC + g * BG + bb] = o[g, bb]
    return out


if __name__ == "__main__":
    rng = np.random.default_rng(0)
    ins = {
        "x": rng.standard_normal((B, T, N, F), np.float32),
        "G": rng.standard_normal((N, N), np.float32) / np.sqrt(N),
        "W_ih": rng.standard_normal((G4, F), np.float32) * 0.05,
        "b_ih": rng.standard_normal((G4,), np.float32) * 0.05,
        "W_hh": rng.standard_normal((G4, H), np.float32) * 0.05,
        "b_hh": rng.standard_normal((G4,), np.float32) * 0.05,
        "W_h1": rng.standard_normal((H, F), np.float32) * 0.05,
        "b_h1": rng.standard_normal((H,), np.float32) * 0.05,
        "W_h2": rng.standard_normal((H, F), np.float32) * 0.05,
        "b_h2": rng.standard_normal((H,), np.float32) * 0.05,
        "W_fc": rng.standard_normal((H, H), np.float32) * 0.05,
        "b_fc": rng.standard_normal((H,), np.float32) * 0.05,
    }
    out = kernel(**ins)
    print("out", out.shape, out.dtype, float(np.abs(out).mean()))


# revision 23
# speedup vs baseline: 1.2367x; 1.0432x over previous
"""Graph-LSTM encoder kernel for 8x Trainium2 NeuronCores.

Problem: B,T,N,F,H = 64,50,24,256,256
    h = graph_linear(G, x0, W_h1, b_h1); c = graph_linear(G, x0, W_h2, b_h2)
    per t: gates = GL(G, x_t, W_ih, b_ih) + GL(G, h, W_hh, b_hh)  (LSTM cell)
    out = tanh(GL(G, h_T, W_fc, b_fc))
where GL(G, x, W, b) = einsum('nm,bmf->bnf', G, x @ W.T) + b
                     = (G . x) @ W.T + b      (mix commutes with projection)

Sharding: data-parallel over batch, 8 batches/core. Per core, batches are
split into 2 groups of 4 (96 rows of 24 nodes each) which pipeline against
each other (PE on one group while ACT/DVE handle the other).

Layouts per group (rows = (batch-in-group, node) on partitions):
  state   h [96,256] bf16, c [96,256] f32 (persistent SBUF)
  mix     mm(lhsT=z[96, fc*128:+128], rhs=BD[96,96]) -> psum[128, fc*128:+96]
          where BD = kron(I4, G^T): block-diag node mix, contracted over rows.
  gates   [128,1024] psum (rows 96+ garbage), accumulated as
            ones[1,128]^T @ bias[1,512]        (bias row, start=True)
          + GzT blocks ^T @ W blocks           (bf16, M=128 for fast LDW)
  cell    gates pre-permuted to [g,i | f,o]: one tanh(g) (can start when
          psum bank0 closes), ONE merged sigmoid over [i,f,o] (768 cols),
          then bf16 DVE cell math. All mix-psum casts run on the idle
          GPSIMD engine so the DVE only does cell arithmetic.

Software pipeline (per loop iteration s, group g = s%2):
  PE:     h-mix(s) | bias+x-gates(s+2) | x-mix(s+4) | h-gates(s)
  GPSIMD: h-mix cast(s), x-mix cast(s+4)
  Sync:   x DMA(s+6)
  ACT/DVE: cell(s)
Each stage consumes data produced >= 2 iterations earlier, so no
intra-iteration cross-engine stalls on the PE queue.

Weights arrive in 2 packed DMAs (init-critical small blob first), not 15.
"""

import sys

sys.path.insert(0, "/opt/trn_rl_repo")

import numpy as np
import ml_dtypes

import concourse.bacc as bacc
import concourse.mybir as mybir
import concourse.tile as tile
from concourse.bass_utils import run_bass_kernel_spmd

B, T, N, F, H = 64, 50, 24, 256, 256
NCORES = 8
B_LOC = B // NCORES      # 8 batches per core
NG = 2                   # pipeline groups per core
BG = B_LOC // NG         # 4 batches per group
R = BG * N               # 96 rows per group
G4 = 4 * H               # 1024 gate width
NSTEP = T * NG

F32 = mybir.dt.float32
BF16 = mybir.dt.bfloat16

LAST_EXEC_NS = None
RUN_KWARGS = {}

# --- packed weight blob column offsets (bf16, [128, *]) ---
# small (init-critical): bd | w12 (w1/w2 interleaved per fc) | ones | b12
C_BD, C_W12, C_ONES, C_B12, C_SMALL = 0, 96, 1120, 1248, 1760
# big: wih | whh | wfc | biasg | bfc
C_WIH, C_WHH, C_WFC, C_BG, C_BFC, C_BIG = 0, 2048, 4096, 4608, 5632, 5888


def _perm_ifog(a, axis=0):
    """[i,f,g,o] -> [i,g,f,o]: bank0 = (i,g) closes first and feeds the
    cell (sig_i, tanh_g, m1) while the PE still fills bank1 = (f,o)."""
    idx = np.concatenate([
        np.arange(0, H),          # i
        np.arange(2 * H, 3 * H),  # g
        np.arange(H, 2 * H),      # f
        np.arange(3 * H, 4 * H),  # o
    ])
    return np.take(a, idx, axis=axis)


def _build_bass():
    nc = bacc.Bacc("TRN2", target_bir_lowering=False, debug=False)

    x_ext = nc.declare_dram_parameter("x", [T, NG, R, F], BF16, isOutput=False)
    ws_ext = nc.declare_dram_parameter("ws", [128, C_SMALL], BF16, isOutput=False)
    wb_ext = nc.declare_dram_parameter("wb", [128, C_BIG], BF16, isOutput=False)
    out_ext = nc.declare_dram_parameter("out", [NG, R, H], F32, isOutput=True)

    with tile.TileContext(nc) as tc:
        with (
            tc.tile_pool(name="wpool", bufs=1) as wpool,
            tc.tile_pool(name="state", bufs=1) as state,
            tc.tile_pool(name="xpool", bufs=4) as xpool,
            tc.tile_pool(name="mixps", bufs=1, space="PSUM") as mixps,
            tc.tile_pool(name="sbx", bufs=4) as sbx,
            tc.tile_pool(name="sbh", bufs=2) as sbh,
            tc.tile_pool(name="gps", bufs=3, space="PSUM") as gps,
            tc.tile_pool(name="ew", bufs=2) as ew,
        ):
            # ---- weights + first x tiles: init-critical DMAs first ----
            ws = wpool.tile([128, C_SMALL], BF16)
            nc.sync.dma_start(ws[:], ws_ext[:])

            xts = {}

            def stage_dma(s):
                xt = xpool.tile([R, F], BF16, tag="xt", name="xt")
                nc.sync.dma_start(xt[:], x_ext[s // NG, s % NG])
                xts[s] = xt

            stage_dma(0)
            stage_dma(1)
            wb = wpool.tile([128, C_BIG], BF16)
            nc.sync.dma_start(wb[:], wb_ext[:])
            stage_dma(2)
            stage_dma(3)

            bd = ws[0:96, C_BD:C_BD + 96]
            w12 = ws[:, C_W12:C_ONES]
            ones = ws[0:1, C_ONES:C_B12]
            b12 = ws[0:1, C_B12:C_SMALL]
            wih = wb[:, C_WIH:C_WHH]
            whh = wb[:, C_WHH:C_WFC]
            wfc = wb[:, C_WFC:C_BG]
            biasg = wb[0:1, C_BG:C_BFC]
            bfc = wb[0:1, C_BFC:C_BIG]

            # ---- persistent state ----
            hs = [state.tile([R, H], BF16, tag=f"h{g}", name=f"h{g}")
                  for g in range(NG)]
            cs = [state.tile([R, H], BF16, tag=f"c{g}", name=f"c{g}")
                  for g in range(NG)]

            msx = {}

            def stage_xmix(s):
                """x-mix for step s: PE mix matmuls + GPSIMD psum->sbuf cast."""
                xt = xts.pop(s)
                ps = mixps.tile([128, 256], F32, tag="mpx", name="mpx")
                for fc in range(2):
                    nc.tensor.matmul(
                        ps[:, fc * 128:fc * 128 + R],
                        xt[:, fc * 128:(fc + 1) * 128],
                        bd, start=True, stop=True)
                sb = sbx.tile([128, 256], BF16, tag="msx", name="msx")
                ps_v = ps[:].rearrange("p (c k) -> p c k", k=128)[:, :, 0:R]
                sb_v = sb[:].rearrange("p (c k) -> p c k", k=128)[:, :, 0:R]
                nc.vector.tensor_copy(sb_v, ps_v)
                msx[s] = sb

            def mix_h(hsb):
                """h-mix: PE matmuls + per-chunk DVE casts (recurrence path).
                Casting each fc chunk separately lets the first fc-major
                h-gate matmuls start a half-cast earlier."""
                ps = mixps.tile([128, 256], F32, tag="mph", name="mph")
                for fc in range(2):
                    nc.tensor.matmul(
                        ps[:, fc * 128:fc * 128 + R],
                        hsb[:, fc * 128:(fc + 1) * 128],
                        bd, start=True, stop=True)
                sb = sbh.tile([128, 256], BF16, tag="msh", name="msh")
                for fc in range(2):
                    nc.vector.tensor_copy(sb[:, fc * 128:fc * 128 + R],
                                          ps[:, fc * 128:fc * 128 + R])
                return sb

            pending = {}

            def stage_bias(s):
                """open step s's gates psum with the bias rows."""
                ps = gps.tile([128, G4], F32, tag="gates", name="gates")
                for nch in range(2):
                    nc.tensor.matmul(ps[:, nch * 512:(nch + 1) * 512],
                                     ones, biasg[:, nch * 512:(nch + 1) * 512],
                                     start=True, stop=False)
                pending[s] = ps

            def stage_xg(s):
                """x-side matmuls into step s's gates psum."""
                gxT = msx.pop(s)
                ps = pending[s]
                for nch in range(2):
                    for fc in range(2):
                        nc.tensor.matmul(
                            ps[:, nch * 512:(nch + 1) * 512],
                            gxT[:, fc * 128:(fc + 1) * 128],
                            wih[:, fc * G4 + nch * 512:fc * G4 + (nch + 1) * 512],
                            start=False, stop=False)

            def close_gates(ps, ghT):
                """h-side matmuls, fc-major: the first two only need the
                first half of the h-mix cast; bank0 = (i,g) stops first."""
                for fc in range(2):
                    for nch in range(2):
                        nc.tensor.matmul(
                            ps[:, nch * 512:(nch + 1) * 512],
                            ghT[:, fc * 128:(fc + 1) * 128],
                            whh[:, fc * G4 + nch * 512:fc * G4 + (nch + 1) * 512],
                            start=False, stop=(fc == 1))

            hmul_pending = {}

            def cell(s):
                """Everything except the final h = sig_o * tanh(c), which is
                issued next iteration (flush_hmul) so the next h-mix cast is
                not stuck behind it in the DVE queue."""
                g = s % NG
                ps = pending.pop(s)
                # bank0 = (i,g): starts as soon as the first psum bank closes
                sig_i = ew.tile([R, H], BF16, tag="sigi", name="sig_i")
                nc.scalar.activation(sig_i[:], ps[0:R, 0:H],
                                     mybir.ActivationFunctionType.Sigmoid)
                tg = ew.tile([R, H], BF16, tag="tg", name="tg")
                nc.scalar.activation(tg[:], ps[0:R, H:2 * H],
                                     mybir.ActivationFunctionType.Tanh)
                m1 = ew.tile([R, H], BF16, tag="m1", name="m1")
                nc.vector.tensor_mul(m1[:], sig_i[:], tg[:])
                # bank1 = (f,o)
                sig_fo = ew.tile([R, 2 * H], BF16, tag="sigfo", name="sig_fo")
                nc.scalar.activation(sig_fo[:], ps[0:R, 2 * H:4 * H],
                                     mybir.ActivationFunctionType.Sigmoid)
                m2 = ew.tile([R, H], BF16, tag="m2", name="m2")
                nc.vector.tensor_mul(m2[:], sig_fo[:, 0:H], cs[g][:])
                nc.vector.tensor_add(cs[g][:], m1[:], m2[:])
                tc_t = ew.tile([R, H], BF16, tag="tc", name="tc")
                nc.scalar.activation(tc_t[:], cs[g][:],
                                     mybir.ActivationFunctionType.Tanh)
                nc.vector.tensor_mul(hs[g][:], sig_fo[:, H:2 * H], tc_t[:])

            def flush_hmul(s):
                pass

            def proj_init(gxT, g):
                """h0/c0 for group g in one psum pass: [w1|w2] packed."""
                ps = gps.tile([128, G4], F32, tag="gates", name="gates")
                nc.tensor.matmul(ps[:, 0:2 * H], ones, b12,
                                 start=True, stop=False)
                for fc in range(2):
                    nc.tensor.matmul(
                        ps[:, 0:2 * H],
                        gxT[:, fc * 128:(fc + 1) * 128],
                        w12[:, fc * 2 * H:(fc + 1) * 2 * H],
                        start=False, stop=(fc == 1))
                nc.vector.tensor_copy(hs[g][:], ps[0:R, 0:H])
                nc.vector.tensor_copy(cs[g][:], ps[0:R, H:2 * H])

            # ---- prologue ----
            # x-mix(0) first, then the PE warm-up (psum from the gates pool)
            # runs back-to-back behind it: the PE stays busy through the
            # big-weight DMA window and the pstate ramp finishes before init.
            stage_xmix(0)
            for i in range(2):
                wu_ps = gps.tile([128, G4], F32, tag="gates", name="wu")
                for _ in range(12):
                    nc.tensor.matmul(wu_ps[0:R, 0:R], bd, bd,
                                     start=True, stop=True)
            stage_xmix(1)
            stage_xmix(2)
            stage_xmix(3)
            # init h0/c0 (reuses the step-0/1 x-mixes; w1/w2 in one pass)
            for g in range(NG):
                proj_init(msx[g], g)
            stage_dma(4)
            stage_dma(5)
            stage_bias(0)
            stage_xg(0)

            # ---- recurrence ----
            for s in range(NSTEP):
                g = s % NG
                ghT = mix_h(hs[g])
                if s + 4 < NSTEP:
                    stage_xmix(s + 4)
                if s + 6 < NSTEP:
                    stage_dma(s + 6)
                if s + 1 < NSTEP:
                    stage_bias(s + 1)
                    stage_xg(s + 1)
                close_gates(pending[s], ghT)
                cell(s)

            # ---- final projection ----
            for g in range(NG):
                ghT = mix_h(hs[g])
                ps = gps.tile([128, G4], F32, tag="gates", name="gates")
                nc.tensor.matmul(ps[:, 0:H], ones, bfc, start=True, stop=False)
                for fc in range(2):
                    nc.tensor.matmul(
                        ps[:, 0:H],
                        ghT[:, fc * 128:(fc + 1) * 128],
                        wfc[:, fc * H:(fc + 1) * H],
                        start=False, stop=(fc == 1))
                o_sb = ew.tile([R, H], F32, tag="osb", name="osb")
                nc.scalar.activation(o_sb[:], ps[0:R, 0:H],
                                     mybir.ActivationFunctionType.Tanh)
                nc.sync.dma_start(out_ext[g], o_sb[:])

    nc.compile()
    return nc


_NC_CACHE = None


def kernel(x, G, W_ih, b_ih, W_hh, b_hh, W_h1, b_h1, W_h2, b_h2, W_fc, b_fc):
    global _NC_CACHE, LAST_EXEC_NS

    x = np.asarray(x)
    G = np.asarray(G, dtype=np.float32)

    # host-side staging
    # x: [B,T,N,F] -> per-core [T, NG, R, F] with b = core*B_LOC + g*BG + bb
    xs = np.asarray(x, dtype=np.float32).reshape(NCORES, NG, BG, T, N, F)
    xs = xs.transpose(0, 3, 1, 2, 4, 5).reshape(NCORES, T, NG, R, F)
    xs = xs.astype(ml_dtypes.bfloat16)

    bd = np.kron(np.eye(BG, dtype=np.float32), G.T)

    def _wt(w):  # [out, in] -> lhs-side [128, 2*out] (feat chunks along cols)
        wt = np.ascontiguousarray(np.asarray(w, np.float32).T)  # [in, out]
        return np.concatenate([wt[0:128], wt[128:256]], axis=1)

    ws = np.zeros((128, C_SMALL), np.float32)
    ws[0:96, C_BD:C_BD + 96] = bd
    w1t, w2t = _wt(W_h1), _wt(W_h2)  # [128, 2H] each, fc chunks along cols
    for fc in range(2):
        ws[:, C_W12 + fc * 512:C_W12 + fc * 512 + 256] = w1t[:, fc * 256:(fc + 1) * 256]
        ws[:, C_W12 + fc * 512 + 256:C_W12 + (fc + 1) * 512] = w2t[:, fc * 256:(fc + 1) * 256]
    ws[0, C_ONES:C_B12] = 1.0
    ws[0, C_B12:C_B12 + 256] = np.asarray(b_h1, np.float32)
    ws[0, C_B12 + 256:C_SMALL] = np.asarray(b_h2, np.float32)

    wbig = np.zeros((128, C_BIG), np.float32)
    wbig[:, C_WIH:C_WHH] = _wt(_perm_ifog(np.asarray(W_ih)))
    wbig[:, C_WHH:C_WFC] = _wt(_perm_ifog(np.asarray(W_hh)))
    wbig[:, C_WFC:C_BG] = _wt(W_fc)
    wbig[0, C_BG:C_BFC] = _perm_ifog(
        np.asarray(b_ih, np.float32) + np.asarray(b_hh, np.float32))
    wbig[0, C_BFC:C_BIG] = np.asarray(b_fc, np.float32)

    ws = ws.astype(ml_dtypes.bfloat16)
    wbig = wbig.astype(ml_dtypes.bfloat16)

    if _NC_CACHE is None:
        _NC_CACHE = _build_bass()
    nc = _NC_CACHE

    in_maps = [dict(x=xs[core], ws=ws, wb=wbig) for core in range(NCORES)]

    res = run_bass_kernel_spmd(nc, in_maps, list(range(NCORES)), **RUN_KWARGS)
    LAST_EXEC_NS = res.exec_time_ns

    out = np.empty((B, N, H), np.float32)
    for core in range(NCORES):
        o = res.results[core]["out"].reshape(NG, BG, N, H)
        for g in range(NG):
            for bb in range(BG):
                out[core * B_LOC + g * BG + bb] = o[g, bb]
    return out


if __name__ == "__main__":
    rng = np.random.default_rng(0)
    ins = {
        "x": rng.standard_normal((B, T, N, F), np.float32),
        "G": rng.standard_normal((N, N), np.float32) / np.sqrt(N),
        "W_ih": rng.standard_normal((G4, F), np.float32) * 0.05,
        "b_ih": rng.standard_normal((G4,), np.float32) * 0.05,
        "W_hh": rng.standard_normal((G4, H), np.float32) * 0.05,
        "b_hh": rng.standard_normal((G4,), np.float32) * 0.05,
        "W_h1": rng.standard_normal((H, F), np.float32) * 0.05,
        "b_h1": rng.standard_normal((H,), np.float32) * 0.05,
        "W_h2": rng.standard_normal((H, F), np.float32) * 0.05,
        "b_h2": rng.standard_normal((H,), np.float32) * 0.05,
        "W_fc": rng.standard_normal((H, H), np.float32) * 0.05,
        "b_fc": rng.standard_normal((H,), np.float32) * 0.05,
    }
    out = kernel(**ins)
    print("out", out.shape, out.dtype, float(np.abs(out).mean()))
